# revision 21
# baseline (speedup 1.0000x reference)
"""Trainium2 Bass kernel for nn_KeypointsLoss.

Math (per batch b):
    x[p,k] = trunc(kp[b,p,k,0] * (W-1)); y likewise from kp[...,1]
    g_row[p,k,h] = exp(-(h-x)^2/(2s^2)) * (vis>0);  g_col[p,k,w] = exp(-(w-y)^2/(2s^2))
    target[k] = sum_p outer(g_row, g_col)            # [H,W]
    per_sample = sum_k |pred[b,k] - target[k]|^2
    loss = sum_b per_sample / (sum(vis[b]) + 1e-6) / B

Strategy (8 cores, data-parallel over B=32 -> 4 batches/core):
  - The tiny 1-D gaussian factor tables (block-diag g_col staircase bands,
    g_row row-tables, k16 tiles) are precomputed on host from the
    keypoints -- derived constants like negi, so every device DMA is
    ready at t=0 and nothing stalls on generated data.
  - pred lands as [hp=96, (k, t, w)] with h = 2*hp + t: each descriptor
    spans two adjacent h-rows.  All three DMA queues stream concurrently
    (the DMA engines are the wall): b1/b3 via gpsimd SWDGE (bf16 cast --
    best bytes-per-packet), b0 via sync HWDGE (f32), b2 via scalar HWDGE
    (f32), chunked so compute consumes each piece as it lands.
  - bf16 batches (negi path): PE splats target (32-row staircase
    matmuls) AND accumulates -pred; ScalarE square-reduces PSUM at
    2 elem/cycle.
  - f32 batches (dve path): PE splats target only; DVE subtracts pred
    from PSUM -> bf16 diff; squares split between ScalarE (b0) and DVE
    affine_mul_reduce (b2).
  - A ones-vector matmul reduces partitions; visibility normalizer and
    final scaling applied host-side to the 8x4 per-batch sums.
"""

import sys
import numpy as np

sys.path.insert(0, "/opt/trn_rl_repo")

B, P, K, H, W = 32, 8, 17, 192, 192
SIGMA = 3.0
INV2S2 = 1.0 / (2.0 * SIGMA**2)
NCORES = 8
NB = B // NCORES          # batches per core
HP = 96                   # h pair index; h = 2*hp + t
KTW = K * 2 * W           # 6528 free cols for the per-batch pred tile
NG = 4                    # full k-groups of 4 (k0..15); k=16 handled separately

_CACHE = {}


def _build():
    import concourse.bass as bass
    import concourse.bacc as bacc
    import concourse.tile as tile
    from concourse import mybir

    f32 = mybir.dt.float32
    bf16 = mybir.dt.bfloat16
    Alu = mybir.AluOpType
    Act = mybir.ActivationFunctionType

    nc = bacc.Bacc("TRN2", target_bir_lowering=False, debug=False,
                   num_devices=NCORES)

    pred_d = nc.dram_tensor("pred", [NB, K, H, W], f32, kind="ExternalInput").ap()
    bd_d = nc.dram_tensor("bd", [128, NB * 4 * W], bf16,
                          kind="ExternalInput").ap()
    growh_d = nc.dram_tensor("growh", [NG, 128, NB * W], bf16,
                             kind="ExternalInput").ap()
    grow1_d = nc.dram_tensor("grow1", [128, W], bf16, kind="ExternalInput").ap()
    gcol1_d = nc.dram_tensor("gcol1", [128, W], bf16, kind="ExternalInput").ap()
    negi_d = nc.dram_tensor("negi", [96, 96], bf16, kind="ExternalInput").ap()
    out_d = nc.dram_tensor("out", [96, NB], f32, kind="ExternalOutput").ap()

    with tile.TileContext(nc) as tc:
        import contextlib
        with contextlib.ExitStack() as ctx:
            consts = ctx.enter_context(tc.tile_pool(name="consts", bufs=1))
            gpool = ctx.enter_context(tc.tile_pool(name="gpool", bufs=1))
            colp = ctx.enter_context(tc.tile_pool(name="cols", bufs=1))
            predp = ctx.enter_context(tc.tile_pool(name="pred", bufs=1))
            scrp = ctx.enter_context(tc.tile_pool(name="scr", bufs=3))
            psump = ctx.enter_context(tc.tile_pool(name="psum", bufs=2, space="PSUM"))

            negi_t = consts.tile([96, 96], bf16, tag="negi")
            ones_t = consts.tile([96, 1], f32, tag="ones")
            accall = consts.tile([96, NB], f32, tag="accall")
            # one stacked staircase tile: every k-band lives on its own
            # rows, so all four k-groups share it; the group selection
            # happens via zeros in the per-group masked grow tables.
            bd_t = consts.tile([128, NB * 4 * W], bf16, tag="bd", name="bd")
            grow_t = [consts.tile([128, NB * W], bf16, tag=f"grow{g}",
                                  name=f"grow{g}") for g in range(NG)]
            grow1 = gpool.tile([128, W], bf16, tag="grow1")
            gcol1 = gpool.tile([128, W], bf16, tag="gcol1")

            nc.vector.memset(ones_t[:], 1.0)

            # pred tiles: [hp, (k, t, w)].  b1/b3 bf16 (SWDGE casts),
            # b0/b2 f32 (HWDGE cannot cast).
            pred_t = [
                predp.tile([HP, KTW], f32, tag="pred0", name="pred0"),
                predp.tile([HP, KTW], bf16, tag="pred1", name="pred1"),
                predp.tile([HP, KTW], f32, tag="pred2", name="pred2"),
                predp.tile([HP, KTW], bf16, tag="pred3", name="pred3"),
            ]

            def pred_dma(eng, b, k0, k1):
                pv4 = pred_t[b][:].rearrange("p (k t w) -> p k t w", t=2, w=W)
                eng.dma_start(
                    out=pv4[:, k0:k1],
                    in_=pred_d[b, k0:k1].rearrange("k (hp t) w -> hp k t w",
                                                   t=2))

            # sync queue: aux tables then b0 quarters
            nc.sync.dma_start(out=bd_t[:], in_=bd_d[:])
            nc.sync.dma_start(out=grow_t[0][:], in_=growh_d[0])
            nc.sync.dma_start(out=grow_t[1][:], in_=growh_d[1])
            pred_dma(nc.sync, 0, 0, 4)
            pred_dma(nc.sync, 0, 4, 8)
            pred_dma(nc.sync, 0, 8, 12)
            pred_dma(nc.sync, 0, 12, K)
            # scalar queue: k16 tables then b2 quarters
            nc.scalar.dma_start(out=negi_t[:], in_=negi_d[:])
            nc.scalar.dma_start(out=grow_t[2][:], in_=growh_d[2])
            nc.scalar.dma_start(out=grow_t[3][:], in_=growh_d[3])
            nc.scalar.dma_start(out=grow1[:], in_=grow1_d[:])
            nc.scalar.dma_start(out=gcol1[:], in_=gcol1_d[:])
            pred_dma(nc.scalar, 2, 0, 4)
            pred_dma(nc.scalar, 2, 4, 8)
            pred_dma(nc.scalar, 2, 8, 12)
            pred_dma(nc.scalar, 2, 12, K)
            # gpsimd SWDGE: the casting batches
            pred_dma(nc.gpsimd, 1, 0, 8)
            pred_dma(nc.gpsimd, 1, 8, K)
            pred_dma(nc.gpsimd, 3, 0, 8)
            pred_dma(nc.gpsimd, 3, 8, K)

            # ---------------- main loop ----------------
            # negi path (b1/b3): PE subtracts pred in PSUM, ScalarE squares
            # PSUM; dve path (b0/b2): DVE subtracts, squares per SQ.
            NEGI = {1: True, 3: True, 0: False, 2: False}
            SQ = {0: "sbuf", 2: "amr"}
            accs_t = [gpool.tile([96, NG + 1], f32, tag=f"accs{b}",
                                 name=f"accs{b}") for b in range(NB)]

            def do_group(b, g):
                pt = pred_t[b]
                gv = grow_t[g][:, b * W:(b + 1) * W].rearrange(
                    "p (h t) -> p h t", t=2)
                ge, go = gv[:, :, 0], gv[:, :, 1]   # [128, 96] stride 2
                pv = pt[:].rearrange("p (k t w) -> p k t w", t=2, w=W)
                bdt = bd_t[:, b * 4 * W:(b + 1) * 4 * W]
                ps = psump.tile([96, 2048], f32, tag="ps", name="ps")
                if NEGI[b]:
                    # k-pairs per half-bank: [0:384) [512:896) [1024:1408)
                    # [1536:1920); PE accumulates -pred on top of the splat
                    nc.tensor.matmul(ps[:, 0:384], ge, bdt[:, 0:384],
                                     start=True, stop=False)
                    nc.tensor.matmul(ps[:, 512:896], ge, bdt[:, 384:768],
                                     start=True, stop=False)
                    nc.tensor.matmul(ps[:, 1024:1408], go, bdt[:, 0:384],
                                     start=True, stop=False)
                    nc.tensor.matmul(ps[:, 1536:1920], go, bdt[:, 384:768],
                                     start=True, stop=False)
                    nc.tensor.matmul(ps[:, 0:384], negi_t[:],
                                     pv[:, 4 * g:4 * g + 2, 0],
                                     start=False, stop=True)
                    nc.tensor.matmul(ps[:, 512:896], negi_t[:],
                                     pv[:, 4 * g + 2:4 * g + 4, 0],
                                     start=False, stop=True)
                    nc.tensor.matmul(ps[:, 1024:1408], negi_t[:],
                                     pv[:, 4 * g:4 * g + 2, 1],
                                     start=False, stop=True)
                    nc.tensor.matmul(ps[:, 1536:1920], negi_t[:],
                                     pv[:, 4 * g + 2:4 * g + 4, 1],
                                     start=False, stop=True)
                    view = ps[:].rearrange("p (a c) -> p a c", c=512)[:, :, 0:384]
                    scr = scrp.tile([96, 1536], bf16, tag="scr", name="scr")
                    sview = scr[:].rearrange("p (a c) -> p a c", c=384)
                    nc.scalar.activation(sview, view, Act.Square,
                                         accum_out=accs_t[b][:, g:g + 1])
                else:
                    # splat only: even [0:768) odd [1024:1792); DVE subtract
                    nc.tensor.matmul(ps[:, 0:512], ge, bdt[:, 0:512],
                                     start=True, stop=True)
                    nc.tensor.matmul(ps[:, 512:768], ge, bdt[:, 512:768],
                                     start=True, stop=True)
                    nc.tensor.matmul(ps[:, 1024:1536], go, bdt[:, 0:512],
                                     start=True, stop=True)
                    nc.tensor.matmul(ps[:, 1536:1792], go, bdt[:, 512:768],
                                     start=True, stop=True)
                    diff = scrp.tile([96, 1536], bf16, tag="diff", name="diff")
                    de = diff[:, 0:768].rearrange("p (a c) -> p a c", c=W)
                    do = diff[:, 768:1536].rearrange("p (a c) -> p a c", c=W)
                    pse = ps[:, 0:768].rearrange("p (a c) -> p a c", c=W)
                    pso = ps[:, 1024:1792].rearrange("p (a c) -> p a c", c=W)
                    nc.vector.tensor_tensor(de, pse, pv[:, 4 * g:4 * g + 4, 0],
                                            Alu.subtract)
                    nc.vector.tensor_tensor(do, pso, pv[:, 4 * g:4 * g + 4, 1],
                                            Alu.subtract)
                    scr = scrp.tile([96, 1536], bf16, tag="scr", name="scr")
                    if SQ[b] == "sbuf":
                        nc.scalar.activation(scr[:], diff[:], Act.Square,
                                             accum_out=accs_t[b][:, g:g + 1])
                    else:
                        nc.vector.affine_mul_reduce(
                            out=scr[:], accum_out=accs_t[b][:, g:g + 1],
                            in0=diff[:], in1=diff[:], scale=1.0, bias=0.0)

            def do_k16(b):
                pt = pred_t[b]
                pv = pt[:].rearrange("p (k t w) -> p k t w", t=2, w=W)
                ps = psump.tile([96, 2048], f32, tag="ps", name="ps")
                g1v = grow1[32 * b:32 * b + P].rearrange("p (h t) -> p h t", t=2)
                gc1 = gcol1[32 * b:32 * b + P, :]
                if NEGI[b]:
                    nc.tensor.matmul(ps[:, 0:192], g1v[:, :, 0], gc1,
                                     start=True, stop=False,
                                     tile_position=(32 * b, 0))
                    nc.tensor.matmul(ps[:, 192:384], g1v[:, :, 1], gc1,
                                     start=True, stop=False,
                                     tile_position=(32 * b, 0))
                    nc.tensor.matmul(ps[:, 0:384], negi_t[:],
                                     pt[:, 16 * 384:17 * 384],
                                     start=False, stop=True)
                    scr = scrp.tile([96, 1536], bf16, tag="scr", name="scr")
                    nc.scalar.activation(scr[:, 0:384], ps[:, 0:384],
                                         Act.Square,
                                         accum_out=accs_t[b][:, NG:NG + 1])
                else:
                    nc.tensor.matmul(ps[:, 0:192], g1v[:, :, 0], gc1,
                                     start=True, stop=True,
                                     tile_position=(32 * b, 0))
                    nc.tensor.matmul(ps[:, 512:704], g1v[:, :, 1], gc1,
                                     start=True, stop=True,
                                     tile_position=(32 * b, 0))
                    diff = scrp.tile([96, 1536], bf16, tag="diff", name="diff")
                    d16 = diff[:, 0:384].rearrange("p (a c) -> p a c", c=W)
                    ps16 = ps[:].rearrange("p (a c) -> p a c", c=512)[:, 0:2, 0:W]
                    nc.vector.tensor_tensor(d16, ps16, pv[:, 16], Alu.subtract)
                    scr = scrp.tile([96, 1536], bf16, tag="scr", name="scr")
                    if SQ[b] == "sbuf":
                        nc.scalar.activation(scr[:, 0:384], diff[:, 0:384],
                                             Act.Square,
                                             accum_out=accs_t[b][:, NG:NG + 1])
                    else:
                        nc.vector.affine_mul_reduce(
                            out=scr[:, 0:384],
                            accum_out=accs_t[b][:, NG:NG + 1],
                            in0=diff[:, 0:384], in1=diff[:, 0:384],
                            scale=1.0, bias=0.0)

            # group-granularity emission in expected data-arrival order;
            # the tail interleaves b0/b2 so DVE subtract/AMR work overlaps
            # the ScalarE squares while b3's last chunk streams in
            SEQ = [(1, 0), (1, 1), (0, 0), (2, 0), (1, 2), (1, 3), (1, "k"),
                   (0, 1), (2, 1), (3, 0), (3, 1), (0, 2), (2, 2),
                   (0, 3), (2, 3), (0, "k"), (2, "k"), (3, 2), (3, 3),
                   (3, "k")]
            for b, g in SEQ:
                if g == "k":
                    do_k16(b)
                    nc.vector.tensor_reduce(accall[:, b:b + 1], accs_t[b][:],
                                            axis=mybir.AxisListType.X,
                                            op=Alu.add)
                else:
                    do_group(b, g)

            # ---------------- finalize: raw per-partition partials --------
            # host sums the 96 partition partials per batch
            nc.gpsimd.dma_start(out=out_d[:], in_=accall[:])

    nc.compile()
    return nc


def get_nc():
    if "nc" not in _CACHE:
        _CACHE["nc"] = _build()
    return _CACHE["nc"]


def make_in_maps(pred_heatmaps, keypoints, visibilities):
    import ml_dtypes
    bf = ml_dtypes.bfloat16
    pred = np.ascontiguousarray(pred_heatmaps, dtype=np.float32)
    kp = np.asarray(keypoints, dtype=np.float32)        # [B,P,K,2]
    vis = np.asarray(visibilities, dtype=np.int32)      # [B,P,K]

    x = np.trunc(kp[..., 0] * (W - 1)).astype(np.int32)  # [B,P,K]
    y = np.trunc(kp[..., 1] * (H - 1)).astype(np.int32)
    valid = ((vis > 0) & (x >= 0) & (x < W) & (y >= 0) & (y < H))
    rng = np.arange(H, dtype=np.float32)
    # g_row centered at x over H; g_col centered at y over W (ref quirk)
    g_row = np.exp(-((rng[None, None, None, :] - x[..., None]) ** 2)
                   * INV2S2).astype(np.float32)          # [B,P,K,H]
    g_row *= valid[..., None]
    g_col = np.exp(-((rng[None, None, None, :] - y[..., None]) ** 2)
                   * INV2S2).astype(np.float32)          # [B,P,K,W]

    negi = (-np.eye(96)).astype(bf)
    in_maps = []
    for c in range(NCORES):
        sl = slice(c * NB, (c + 1) * NB)
        gr = g_row[sl]    # [NB,P,K,H]
        gc = g_col[sl]    # [NB,P,K,W]
        # stacked staircase: bd[8k+p, b, (k%4)*W+w] = g_col[b,p,k,w]
        bd = np.zeros((128, NB, 4 * W), dtype=np.float32)
        # masked grow: growh[g, 8k+p, b*W+h] = g_row[b,p,k,h] iff k//4==g
        growh = np.zeros((NG, 128, NB, W), dtype=np.float32)
        for k in range(16):
            j = k % 4
            bd[8 * k:8 * k + 8, :, j * W:(j + 1) * W] = \
                gc[:, :, k, :].transpose(1, 0, 2)
            growh[k // 4, 8 * k:8 * k + 8] = \
                gr[:, :, k, :].transpose(1, 0, 2)
        g16r = np.zeros((128, W), dtype=np.float32)
        g16c = np.zeros((128, W), dtype=np.float32)
        for b in range(NB):
            g16r[32 * b:32 * b + P, :] = gr[b, :, 16, :]
            g16c[32 * b:32 * b + P, :] = gc[b, :, 16, :]
        in_maps.append({
            "pred": pred[sl],
            "bd": bd.reshape(128, NB * 4 * W).astype(bf),
            "growh": growh.reshape(NG, 128, NB * W).astype(bf),
            "grow1": g16r.astype(bf),
            "gcol1": g16c.astype(bf),
            "negi": negi,
        })
    return in_maps


def kernel(pred_heatmaps, keypoints, visibilities):
    from concourse.bass_utils import run_bass_kernel_spmd

    nc = get_nc()
    in_maps = make_in_maps(pred_heatmaps, keypoints, visibilities)
    res = run_bass_kernel_spmd(nc, in_maps, core_ids=list(range(NCORES)))
    vis = np.asarray(visibilities, dtype=np.float64)     # [B,P,K]
    den = vis.reshape(B, -1).sum(axis=1) + 1e-6
    total = np.float64(0.0)
    for c in range(NCORES):
        sums = np.asarray(res.results[c]["out"], dtype=np.float64)
        sums = sums.reshape(96, NB).sum(axis=0)
        total += (sums / den[c * NB:(c + 1) * NB]).sum()
    return np.float32(total / B)


# revision 27
# speedup vs baseline: 1.1859x; 1.1859x over previous
"""Trainium2 Bass kernel for nn_KeypointsLoss.

Math (per batch b):
    x[p,k] = trunc(kp[b,p,k,0] * (W-1)); y likewise from kp[...,1]
    g_row[p,k,h] = exp(-(h-x)^2/(2s^2)) * (vis>0);  g_col[p,k,w] = exp(-(w-y)^2/(2s^2))
    target[k] = sum_p outer(g_row, g_col)            # [H,W]
    per_sample = sum_k |pred[b,k] - target[k]|^2
    loss = sum_b per_sample / (sum(vis[b]) + 1e-6) / B

Strategy (8 cores, data-parallel over B=32 -> 4 batches/core):
  - The tiny 1-D gaussian factor tables (block-diag g_col staircase bands,
    g_row row-tables, k16 tiles) are precomputed on host from the
    keypoints -- derived constants like negi, so every device DMA is
    ready at t=0 and nothing stalls on generated data.
  - pred lands as [hp=96, (k, t, w)] with h = 2*hp + t: each descriptor
    spans two adjacent h-rows.  All three DMA queues stream concurrently
    (the DMA engines are the wall): b1/b3 via gpsimd SWDGE (bf16 cast --
    best bytes-per-packet), b0 via sync HWDGE (f32), b2 via scalar HWDGE
    (f32), chunked so compute consumes each piece as it lands.
  - bf16 batches (negi path): PE splats target (32-row staircase
    matmuls) AND accumulates -pred; ScalarE square-reduces PSUM at
    2 elem/cycle.
  - f32 batches (dve path): PE splats target only; DVE subtracts pred
    from PSUM -> bf16 diff; squares split between ScalarE (b0) and DVE
    affine_mul_reduce (b2).
  - A ones-vector matmul reduces partitions; visibility normalizer and
    final scaling applied host-side to the 8x4 per-batch sums.
"""

import sys
import numpy as np

sys.path.insert(0, "/opt/trn_rl_repo")

B, P, K, H, W = 32, 8, 17, 192, 192
SIGMA = 3.0
INV2S2 = 1.0 / (2.0 * SIGMA**2)
NCORES = 8
NB = B // NCORES          # batches per core
HP = 96                   # h pair index; h = 2*hp + t
KTW = K * 2 * W           # 6528 free cols for the per-batch pred tile
NG = 4                    # full k-groups of 4 (k0..15); k=16 handled separately

_CACHE = {}


def _build():
    import concourse.bass as bass
    import concourse.bacc as bacc
    import concourse.tile as tile
    from concourse import mybir

    f32 = mybir.dt.float32
    bf16 = mybir.dt.bfloat16
    Alu = mybir.AluOpType
    Act = mybir.ActivationFunctionType

    nc = bacc.Bacc("TRN2", target_bir_lowering=False, debug=False,
                   num_devices=NCORES)

    pred_d = nc.dram_tensor("pred", [NB, K, H, W], f32, kind="ExternalInput").ap()
    bd_d = nc.dram_tensor("bd", [NG, 32, NB * 4 * W], bf16,
                          kind="ExternalInput").ap()
    growh_d = nc.dram_tensor("growh", [128, NB * W], bf16,
                             kind="ExternalInput").ap()
    grow1_d = nc.dram_tensor("grow1", [128, W], bf16, kind="ExternalInput").ap()
    gcol1_d = nc.dram_tensor("gcol1", [128, W], bf16, kind="ExternalInput").ap()
    negi_d = nc.dram_tensor("negi", [96, 96], bf16, kind="ExternalInput").ap()
    out_d = nc.dram_tensor("out", [96, NB], f32, kind="ExternalOutput").ap()

    with tile.TileContext(nc) as tc:
        import contextlib
        with contextlib.ExitStack() as ctx:
            consts = ctx.enter_context(tc.tile_pool(name="consts", bufs=1))
            gpool = ctx.enter_context(tc.tile_pool(name="gpool", bufs=1))
            colp = ctx.enter_context(tc.tile_pool(name="cols", bufs=1))
            predp = ctx.enter_context(tc.tile_pool(name="pred", bufs=1))
            scrp = ctx.enter_context(tc.tile_pool(name="scr", bufs=3))
            psump = ctx.enter_context(tc.tile_pool(name="psum", bufs=2, space="PSUM"))

            negi_t = consts.tile([96, 96], bf16, tag="negi")
            ones_t = consts.tile([96, 1], f32, tag="ones")
            accall = consts.tile([96, NB], f32, tag="accall")
            # per-group staircase tiles: host-built bands land on rows
            # [32g:32g+32) of DVE-zeroed tiles (the DMA is band-only to
            # keep aux bytes small -- the kernel is DMA-bound)
            bd_g = [consts.tile([128, NB * 4 * W], bf16, tag=f"bd_g{g}",
                                name=f"bd_g{g}") for g in range(NG)]
            grow_all = consts.tile([128, NB * W], bf16, tag="grow_all",
                                   name="grow_all")
            for g in range(NG):
                nc.vector.memset(bd_g[g][:].bitcast(f32), 0.0)
            grow1 = gpool.tile([128, W], bf16, tag="grow1")
            gcol1 = gpool.tile([128, W], bf16, tag="gcol1")

            nc.vector.memset(ones_t[:], 1.0)

            # pred tiles: [hp, (k, t, w)].  b1/b3 bf16 (SWDGE casts),
            # b0/b2 f32 (HWDGE cannot cast).
            pred_t = [
                predp.tile([HP, KTW], f32, tag="pred0", name="pred0"),
                predp.tile([HP, KTW], bf16, tag="pred1", name="pred1"),
                predp.tile([HP, KTW], f32, tag="pred2", name="pred2"),
                predp.tile([HP, KTW], bf16, tag="pred3", name="pred3"),
            ]

            def pred_dma(eng, b, k0, k1):
                pv4 = pred_t[b][:].rearrange("p (k t w) -> p k t w", t=2, w=W)
                eng.dma_start(
                    out=pv4[:, k0:k1],
                    in_=pred_d[b, k0:k1].rearrange("k (hp t) w -> hp k t w",
                                                   t=2))

            # sync queue: aux tables then b0 quarters
            nc.sync.dma_start(out=grow_all[:], in_=growh_d[:])
            for g in range(NG):
                nc.sync.dma_start(out=bd_g[g][32 * g:32 * g + 32, :],
                                  in_=bd_d[g])
            pred_dma(nc.sync, 0, 0, 4)
            pred_dma(nc.sync, 0, 4, 8)
            pred_dma(nc.sync, 0, 8, 12)
            pred_dma(nc.sync, 0, 12, K)
            # scalar queue: k16 tables then b2 quarters
            nc.scalar.dma_start(out=negi_t[:], in_=negi_d[:])
            nc.scalar.dma_start(out=grow1[:], in_=grow1_d[:])
            nc.scalar.dma_start(out=gcol1[:], in_=gcol1_d[:])
            pred_dma(nc.scalar, 2, 0, 4)
            pred_dma(nc.scalar, 2, 4, 8)
            pred_dma(nc.scalar, 2, 8, 12)
            pred_dma(nc.scalar, 2, 12, K)
            # gpsimd SWDGE: the casting batches
            pred_dma(nc.gpsimd, 1, 0, 8)
            pred_dma(nc.gpsimd, 1, 8, K)
            pred_dma(nc.gpsimd, 3, 0, 8)
            pred_dma(nc.gpsimd, 3, 8, K)

            # ---------------- main loop ----------------
            # negi path (b1/b3): PE subtracts pred in PSUM, ScalarE squares
            # PSUM; dve path (b0/b2): DVE subtracts, squares per SQ.
            NEGI = {1: True, 3: True, 0: False, 2: False}
            SQ = {0: "sbuf", 2: "amr"}
            accs_t = [gpool.tile([96, NG + 1], f32, tag=f"accs{b}",
                                 name=f"accs{b}") for b in range(NB)]

            def do_group(b, g):
                pt = pred_t[b]
                gv = grow_all[:, b * W:(b + 1) * W].rearrange(
                    "p (h t) -> p h t", t=2)
                ge, go = gv[:, :, 0], gv[:, :, 1]   # [128, 96] stride 2
                pv = pt[:].rearrange("p (k t w) -> p k t w", t=2, w=W)
                bdt = bd_g[g][:, b * 4 * W:(b + 1) * 4 * W]
                ps = psump.tile([96, 2048], f32, tag="ps", name="ps")
                if NEGI[b]:
                    # k-pairs per half-bank: [0:384) [512:896) [1024:1408)
                    # [1536:1920); PE accumulates -pred on top of the splat
                    nc.tensor.matmul(ps[:, 0:384], ge, bdt[:, 0:384],
                                     start=True, stop=False)
                    nc.tensor.matmul(ps[:, 512:896], ge, bdt[:, 384:768],
                                     start=True, stop=False)
                    nc.tensor.matmul(ps[:, 1024:1408], go, bdt[:, 0:384],
                                     start=True, stop=False)
                    nc.tensor.matmul(ps[:, 1536:1920], go, bdt[:, 384:768],
                                     start=True, stop=False)
                    nc.tensor.matmul(ps[:, 0:384], negi_t[:],
                                     pv[:, 4 * g:4 * g + 2, 0],
                                     start=False, stop=True)
                    nc.tensor.matmul(ps[:, 512:896], negi_t[:],
                                     pv[:, 4 * g + 2:4 * g + 4, 0],
                                     start=False, stop=True)
                    nc.tensor.matmul(ps[:, 1024:1408], negi_t[:],
                                     pv[:, 4 * g:4 * g + 2, 1],
                                     start=False, stop=True)
                    nc.tensor.matmul(ps[:, 1536:1920], negi_t[:],
                                     pv[:, 4 * g + 2:4 * g + 4, 1],
                                     start=False, stop=True)
                    view = ps[:].rearrange("p (a c) -> p a c", c=512)[:, :, 0:384]
                    scr = scrp.tile([96, 1536], bf16, tag="scr", name="scr")
                    sview = scr[:].rearrange("p (a c) -> p a c", c=384)
                    nc.scalar.activation(sview, view, Act.Square,
                                         accum_out=accs_t[b][:, g:g + 1])
                else:
                    # splat only: even [0:768) odd [1024:1792); DVE subtract
                    nc.tensor.matmul(ps[:, 0:512], ge, bdt[:, 0:512],
                                     start=True, stop=True)
                    nc.tensor.matmul(ps[:, 512:768], ge, bdt[:, 512:768],
                                     start=True, stop=True)
                    nc.tensor.matmul(ps[:, 1024:1536], go, bdt[:, 0:512],
                                     start=True, stop=True)
                    nc.tensor.matmul(ps[:, 1536:1792], go, bdt[:, 512:768],
                                     start=True, stop=True)
                    diff = scrp.tile([96, 1536], bf16, tag="diff", name="diff")
                    de = diff[:, 0:768].rearrange("p (a c) -> p a c", c=W)
                    do = diff[:, 768:1536].rearrange("p (a c) -> p a c", c=W)
                    pse = ps[:, 0:768].rearrange("p (a c) -> p a c", c=W)
                    pso = ps[:, 1024:1792].rearrange("p (a c) -> p a c", c=W)
                    nc.vector.tensor_tensor(de, pse, pv[:, 4 * g:4 * g + 4, 0],
                                            Alu.subtract)
                    nc.vector.tensor_tensor(do, pso, pv[:, 4 * g:4 * g + 4, 1],
                                            Alu.subtract)
                    scr = scrp.tile([96, 1536], bf16, tag="scr", name="scr")
                    if SQ[b] == "sbuf":
                        nc.scalar.activation(scr[:], diff[:], Act.Square,
                                             accum_out=accs_t[b][:, g:g + 1])
                    else:
                        nc.vector.affine_mul_reduce(
                            out=scr[:], accum_out=accs_t[b][:, g:g + 1],
                            in0=diff[:], in1=diff[:], scale=1.0, bias=0.0)

            def do_k16(b):
                pt = pred_t[b]
                pv = pt[:].rearrange("p (k t w) -> p k t w", t=2, w=W)
                ps = psump.tile([96, 2048], f32, tag="ps", name="ps")
                g1v = grow1[32 * b:32 * b + P].rearrange("p (h t) -> p h t", t=2)
                gc1 = gcol1[32 * b:32 * b + P, :]
                if NEGI[b]:
                    nc.tensor.matmul(ps[:, 0:192], g1v[:, :, 0], gc1,
                                     start=True, stop=False,
                                     tile_position=(32 * b, 0))
                    nc.tensor.matmul(ps[:, 192:384], g1v[:, :, 1], gc1,
                                     start=True, stop=False,
                                     tile_position=(32 * b, 0))
                    nc.tensor.matmul(ps[:, 0:384], negi_t[:],
                                     pt[:, 16 * 384:17 * 384],
                                     start=False, stop=True)
                    scr = scrp.tile([96, 1536], bf16, tag="scr", name="scr")
                    nc.scalar.activation(scr[:, 0:384], ps[:, 0:384],
                                         Act.Square,
                                         accum_out=accs_t[b][:, NG:NG + 1])
                else:
                    nc.tensor.matmul(ps[:, 0:192], g1v[:, :, 0], gc1,
                                     start=True, stop=True,
                                     tile_position=(32 * b, 0))
                    nc.tensor.matmul(ps[:, 512:704], g1v[:, :, 1], gc1,
                                     start=True, stop=True,
                                     tile_position=(32 * b, 0))
                    diff = scrp.tile([96, 1536], bf16, tag="diff", name="diff")
                    d16 = diff[:, 0:384].rearrange("p (a c) -> p a c", c=W)
                    ps16 = ps[:].rearrange("p (a c) -> p a c", c=512)[:, 0:2, 0:W]
                    nc.vector.tensor_tensor(d16, ps16, pv[:, 16], Alu.subtract)
                    scr = scrp.tile([96, 1536], bf16, tag="scr", name="scr")
                    if SQ[b] == "sbuf":
                        nc.scalar.activation(scr[:, 0:384], diff[:, 0:384],
                                             Act.Square,
                                             accum_out=accs_t[b][:, NG:NG + 1])
                    else:
                        nc.vector.affine_mul_reduce(
                            out=scr[:, 0:384],
                            accum_out=accs_t[b][:, NG:NG + 1],
                            in0=diff[:, 0:384], in1=diff[:, 0:384],
                            scale=1.0, bias=0.0)

            # group-granularity emission in expected data-arrival order;
            # the tail interleaves b0/b2 so DVE subtract/AMR work overlaps
            # the ScalarE squares while b3's last chunk streams in
            SEQ = [(1, 0), (1, 1), (0, 0), (2, 0), (1, 2), (1, 3), (1, "k"),
                   (0, 1), (2, 1), (3, 0), (3, 1), (0, 2), (2, 2),
                   (0, 3), (2, 3), (0, "k"), (2, "k"), (3, 2), (3, 3),
                   (3, "k")]
            for b, g in SEQ:
                if g == "k":
                    do_k16(b)
                    nc.vector.tensor_reduce(accall[:, b:b + 1], accs_t[b][:],
                                            axis=mybir.AxisListType.X,
                                            op=Alu.add)
                else:
                    do_group(b, g)

            # ---------------- finalize: raw per-partition partials --------
            # host sums the 96 partition partials per batch
            nc.gpsimd.dma_start(out=out_d[:], in_=accall[:])

    nc.compile()
    return nc


def get_nc():
    if "nc" not in _CACHE:
        _CACHE["nc"] = _build()
    return _CACHE["nc"]


def make_in_maps(pred_heatmaps, keypoints, visibilities):
    import ml_dtypes
    bf = ml_dtypes.bfloat16
    pred = np.ascontiguousarray(pred_heatmaps, dtype=np.float32)
    kp = np.asarray(keypoints, dtype=np.float32)        # [B,P,K,2]
    vis = np.asarray(visibilities, dtype=np.int32)      # [B,P,K]

    x = np.trunc(kp[..., 0] * (W - 1)).astype(np.int32)  # [B,P,K]
    y = np.trunc(kp[..., 1] * (H - 1)).astype(np.int32)
    valid = ((vis > 0) & (x >= 0) & (x < W) & (y >= 0) & (y < H))
    rng = np.arange(H, dtype=np.float32)
    # g_row centered at x over H; g_col centered at y over W (ref quirk)
    g_row = np.exp(-((rng[None, None, None, :] - x[..., None]) ** 2)
                   * INV2S2).astype(np.float32)          # [B,P,K,H]
    g_row *= valid[..., None]
    g_col = np.exp(-((rng[None, None, None, :] - y[..., None]) ** 2)
                   * INV2S2).astype(np.float32)          # [B,P,K,W]

    negi = (-np.eye(96)).astype(bf)
    in_maps = []
    for c in range(NCORES):
        sl = slice(c * NB, (c + 1) * NB)
        gr = g_row[sl]    # [NB,P,K,H]
        gc = g_col[sl]    # [NB,P,K,W]
        # staircase bands: bd[g, 8j+p, b, j*W+w] = g_col[b,p,4g+j,w]
        bd = np.zeros((NG, 32, NB, 4 * W), dtype=np.float32)
        # grow rows: growh[8k+p, b*W+h] = g_row[b,p,k,h] for k<16
        growh = np.zeros((128, NB, W), dtype=np.float32)
        for k in range(16):
            j = k % 4
            bd[k // 4, 8 * j:8 * j + 8, :, j * W:(j + 1) * W] = \
                gc[:, :, k, :].transpose(1, 0, 2)
            growh[8 * k:8 * k + 8] = \
                gr[:, :, k, :].transpose(1, 0, 2)
        g16r = np.zeros((128, W), dtype=np.float32)
        g16c = np.zeros((128, W), dtype=np.float32)
        for b in range(NB):
            g16r[32 * b:32 * b + P, :] = gr[b, :, 16, :]
            g16c[32 * b:32 * b + P, :] = gc[b, :, 16, :]
        in_maps.append({
            "pred": pred[sl],
            "bd": bd.reshape(NG, 32, NB * 4 * W).astype(bf),
            "growh": growh.reshape(128, NB * W).astype(bf),
            "grow1": g16r.astype(bf),
            "gcol1": g16c.astype(bf),
            "negi": negi,
        })
    return in_maps


def kernel(pred_heatmaps, keypoints, visibilities):
    from concourse.bass_utils import run_bass_kernel_spmd

    nc = get_nc()
    in_maps = make_in_maps(pred_heatmaps, keypoints, visibilities)
    res = run_bass_kernel_spmd(nc, in_maps, core_ids=list(range(NCORES)))
    vis = np.asarray(visibilities, dtype=np.float64)     # [B,P,K]
    den = vis.reshape(B, -1).sum(axis=1) + 1e-6
    total = np.float64(0.0)
    for c in range(NCORES):
        sums = np.asarray(res.results[c]["out"], dtype=np.float64)
        sums = sums.reshape(96, NB).sum(axis=0)
        total += (sums / den[c * NB:(c + 1) * NB]).sum()
    return np.float32(total / B)
